# revision 49
# baseline (speedup 1.0000x reference)
"""Masked multi-head attention (B=8, N=1024, C=768, H=12) on 8 trn2 NeuronCores.

Sharding: pure data-parallel over batch - core i computes batch element i
end-to-end (qkv linear, masked softmax attention, output projection).
No collectives.

Device-side layout is fully "transposed attention":
  qkT   [2C, N]  (q/k heads as [D, N] blocks, produced directly by matmul)
  S.T   [m, n] per head (keys on partitions)  ->  softmax over partition dim
        handled with: no max-subtraction (scores are O(1)), row-sums via a
        ones-column appended to V in the P@V matmul.
  EV.T  [D+1, N] per head accumulated in PSUM; row D holds the softmax denom.
Host pre-transposes/casts x, mask, qkv_w, proj_w (layout prep for the chosen
sharding); all matmuls run in bf16 with fp32 PSUM accumulation.
"""

import numpy as np
import ml_dtypes

import concourse.bass as bass
import concourse.mybir as mybir
import concourse.tile as tile
from concourse import bacc
from concourse.bass_utils import run_bass_kernel_spmd

B, N, C, H = 8, 1024, 768, 12
D = C // H  # 64
SCALE = 0.125
NT = N // 128  # 8 n-tiles
CT = C // 128  # 6 c-tiles
BF16 = mybir.dt.bfloat16
F32 = mybir.dt.float32
NPBF16 = ml_dtypes.bfloat16

_CACHE: dict = {}


def _build_bass():
    nc = bacc.Bacc(None, target_bir_lowering=False, debug=False)

    xT_d = nc.dram_tensor("xT", [C, N], BF16, kind="ExternalInput")
    maskT_d = nc.dram_tensor("maskT", [N, N], BF16, kind="ExternalInput")
    wT_d = nc.dram_tensor("qkv_wT", [C, 3 * C], BF16, kind="ExternalInput")
    pwT_d = nc.dram_tensor("proj_wT", [C, C], BF16, kind="ExternalInput")
    qb_d = nc.dram_tensor("qb_col", [128, 2 * C // 128], F32, kind="ExternalInput")
    vb_d = nc.dram_tensor("vb_row", [1, C], BF16, kind="ExternalInput")
    pb_d = nc.dram_tensor("pb_row", [1, C], BF16, kind="ExternalInput")
    ones_d = nc.dram_tensor("ones_row", [1, 128], BF16, kind="ExternalInput")
    ind_d = nc.dram_tensor("ind", [34, C], BF16, kind="ExternalInput")
    out_d = nc.dram_tensor("out", [N, C], F32, kind="ExternalOutput")

    with tile.TileContext(nc) as tc:
        _emit(nc, tc, xT_d, maskT_d, wT_d, pwT_d, qb_d, vb_d, pb_d,
              ones_d, ind_d, out_d)
    nc.compile()
    return nc


def _emit(nc, tc, xT_d, maskT_d, wT_d, pwT_d, qb_d, vb_d, pb_d,
          ones_d, ind_d, out_d):
    Exp = mybir.ActivationFunctionType.Exp
    Ident = mybir.ActivationFunctionType.Identity

    with (
        tc.tile_pool(name="consts", bufs=1) as consts,
        tc.tile_pool(name="work", bufs=6) as work,
    ):
        # ---- persistent SBUF residents -------------------------------
        xT = consts.tile([128, CT, N], BF16, name="xT_sb")
        wT = consts.tile([128, CT, 3 * C], BF16, name="wT_sb")
        pwT = consts.tile([128, CT, C], BF16, name="pwT_sb")
        maskT = consts.tile([128, NT, N], BF16, name="maskT_sb")
        qb = consts.tile([128, 2 * C // 128], F32, name="qb_sb")
        vb = consts.tile([1, C], BF16, name="vb_sb")
        pb = consts.tile([1, C], BF16, name="pb_sb")
        ones = consts.tile([1, 128], BF16, name="ones_sb")
        qkT = consts.tile([128, 2 * C // 128, N], BF16, name="qkT_sb")
        v_ext = consts.tile([128, NT, H * (D + 1)], BF16, name="vext_sb")
        attnT = consts.tile([128, CT, N], BF16, name="attnT_sb")
        ind = consts.tile([34, C], BF16, name="ind_sb")
        rs12 = consts.tile([34, N], F32, name="rs12_sb")
        rs80 = consts.tile([80, 128], F32, name="rs80_sb")
        rc80 = consts.tile([80, 128], F32, name="rc80_sb")
        rs16 = consts.tile([16, 128], F32, name="rs16_sb")
        rc16 = consts.tile([16, 128], F32, name="rc16_sb")
        rc12b = consts.tile([34, N], BF16, name="rc12_sb")

        # split loads, issued in consumption order so compute starts early
        xT_r = xT_d.ap().rearrange("(t p) n -> p t n", p=128)
        wT_r = wT_d.ap().rearrange("(t p) n -> p t n", p=128)
        maskT_r = maskT_d.ap().rearrange("(t p) n -> p t n", p=128)
        pwT_r = pwT_d.ap().rearrange("(t p) n -> p t n", p=128)
        # tiny const loads on the ACT HWDGE ring, big loads on the SP ring
        nc.scalar.dma_start(out=qb, in_=qb_d.ap())
        nc.scalar.dma_start(out=vb, in_=vb_d.ap())
        nc.scalar.dma_start(out=ones, in_=ones_d.ap())
        nc.scalar.dma_start(out=pb, in_=pb_d.ap())
        nc.scalar.dma_start(out=ind, in_=ind_d.ap())
        for ct in range(CT):
            nc.sync.dma_start(out=wT[:, ct, :], in_=wT_r[:, ct, :])
            nc.sync.dma_start(out=xT[:, ct, :], in_=xT_r[:, ct, :])
        for j in range(NT):
            nc.sync.dma_start(out=maskT[:, j, :], in_=maskT_r[:, j, :])
        for ct in range(CT):
            nc.sync.dma_start(out=pwT[:, ct, :], in_=pwT_r[:, ct, :])

        nc.vector.memset(rc12b, 0.0)

        # ones columns of v_ext (col D of each head block)
        v_ext_h = v_ext.rearrange("p t (h e) -> p t h e", e=D + 1)
        nc.vector.memset(v_ext_h[:, :, :, D:D + 1], 1.0)

        with (
            tc.tile_pool(name="rsr", bufs=2) as rsr,
            tc.tile_pool(name="psQV", bufs=1, space="PSUM") as psQV,
            tc.tile_pool(name="psST", bufs=2, space="PSUM") as psST,
            tc.tile_pool(name="psEV", bufs=1, space="PSUM") as psEV,
        ):
            def emit_qk_tile(i, pool=None, tag="qv"):
                # qkT rows i*128..: q rows for i<6, k rows for i>=6
                psQ = (pool or psQV).tile([128, N], F32, name="psQ", tag=tag)
                for ct in range(CT):
                    lhsT = wT[:, ct, i * 128:(i + 1) * 128]
                    for half in range(2):
                        nc.tensor.matmul(
                            psQ[:, half * 512:(half + 1) * 512],
                            lhsT,
                            xT[:, ct, half * 512:(half + 1) * 512],
                            start=(ct == 0),
                            stop=(ct == CT - 1),
                        )
                if i < CT:
                    nc.scalar.activation(
                        out=qkT[:, i, :], in_=psQ[:, 0:N], func=Ident,
                        bias=qb[:, i:i + 1], scale=1.0,
                    )
                else:
                    nc.vector.tensor_scalar_add(
                        out=qkT[:, i, :], in0=psQ[:, 0:N],
                        scalar1=qb[:, i:i + 1],
                    )

            def emit_v_tile(nt):
                psV = psQV.tile([128, N], F32, name="psV", tag="qv")
                for ct in range(CT):
                    lhsT = xT[:, ct, nt * 128:(nt + 1) * 128]
                    nc.tensor.matmul(
                        psV[:, 0:512], lhsT, wT[:, ct, 2 * C:2 * C + 512],
                        start=(ct == 0), stop=False,
                    )
                    nc.tensor.matmul(
                        psV[:, 512:768], lhsT, wT[:, ct, 2 * C + 512:3 * C],
                        start=(ct == 0), stop=False,
                    )
                nc.tensor.matmul(psV[:, 0:512], ones, vb[:, 0:512],
                                 start=False, stop=True)
                nc.tensor.matmul(psV[:, 512:768], ones, vb[:, 512:768],
                                 start=False, stop=True)
                nc.vector.tensor_copy(
                    out=v_ext_h[:, nt, :, 0:D],
                    in_=psV[:, 0:C].rearrange("p (h d) -> p h d", d=D),
                )

            def emit_head(h, rsl):
                po = (h % 2) * 64  # partition offset of this head's d-rows
                qt = h // 2        # q tile index; k tile = 6 + qt
                psE = psEV.tile([D + 1, N], F32, name="psE", tag="psE")
                for j in range(NT):
                    if h == 0:
                        emit_v_tile(j)
                    if h % 2 == 1 and h + 2 < H and j == NT - 3:
                        # next pair's k tile, emitted a few j-steps early so
                        # its evacuation clears the DVE queue before the
                        # next head's first S matmul needs it
                        emit_qk_tile(CT + h // 2 + 1)
                    psS = psST.tile([128, N], F32, name="psS", tag="psS")
                    kT_ap = qkT[po:po + D, CT + qt, j * 128:(j + 1) * 128]
                    for half in range(2):
                        nc.tensor.matmul(
                            psS[:, half * 512:(half + 1) * 512],
                            kT_ap,
                            qkT[po:po + D, qt, half * 512:(half + 1) * 512],
                            start=True, stop=True,
                        )
                    e_sb = work.tile([128, N], BF16, name="e_sb", tag="e_sb")
                    nc.scalar.activation(out=e_sb, in_=psS, func=Exp,
                                         scale=SCALE)
                    em_sb = work.tile([128, N], BF16, name="em_sb",
                                      tag="em_sb")
                    nc.vector.tensor_mul(out=em_sb, in0=e_sb,
                                         in1=maskT[:, j, :])
                    v_ap = v_ext[:, j, h * (D + 1):(h + 1) * (D + 1)]
                    for half in range(2):
                        nc.tensor.matmul(
                            psE[:, half * 512:(half + 1) * 512],
                            v_ap,
                            em_sb[:, half * 512:(half + 1) * 512],
                            start=(j == 0), stop=(j == NT - 1),
                        )
                # evacuate: EV rows -> attnT (bf16), denom row -> rs (fp32)
                nc.vector.tensor_copy(out=attnT[po:po + D, qt, :],
                                      in_=psE[0:D, :])
                nc.vector.tensor_copy(out=rsl[:, h % 2, :],
                                      in_=psE[D:D + 1, :])

            def emit_norm(ct, pool, tag):
                # broadcast the pair's reciprocal rows onto 2x64 partitions
                # via an indicator matmul, then scale attnT in place
                psr = pool.tile([128, N], F32, name="psr", tag=tag)
                for half in range(2):
                    nc.tensor.matmul(
                        psr[:, half * 512:(half + 1) * 512],
                        ind[:, ct * 128:(ct + 1) * 128],
                        rc12b[:, half * 512:(half + 1) * 512],
                        start=True, stop=True,
                    )
                nc.vector.tensor_mul(out=attnT[:, ct, :], in0=attnT[:, ct, :],
                                     in1=psr)

            def emit_pair_recip(hp, rsl):
                # spread the pair's two denominator rows (partition 0) onto
                # partitions 2hp,2hp+1 of rs12 (pair 5 goes to partitions
                # 32,33 so its reciprocal is a legal standalone ACT op)
                ro = 2 * hp if hp < 5 else 32
                nc.sync.dma_start(out=rs12[ro:ro + 2, :], in_=rsl)
                if hp == 4:
                    # heads 0..9 are all in rs12: reciprocal on DVE at 8x
                    # lane efficiency by spreading [10, N] onto [80, 128]
                    # (SBUF->SBUF partition-spread DMA; same flat order).
                    # No ACT involvement -> no activation-table switches.
                    nc.sync.dma_start(out=rs80, in_=rs12[0:10, :])
                    nc.vector.reciprocal(out=rc80, in_=rs80)
                    nc.gpsimd.dma_start(out=rc12b[0:10, :], in_=rc80)

            # pair 0's qk tiles first, then heads with the next pair's qk
            # tiles emitted between the two heads of each pair so the
            # scheduler can fill attention's ACT-gated PE gaps with them
            # pair-0 qk tiles borrow the two (still idle) ST slots so they
            # pipeline with the arriving weight/x loads
            emit_qk_tile(0, pool=psST, tag="psS")
            emit_qk_tile(CT, pool=psST, tag="psS")
            for h in range(H):
                if h % 2 == 0:
                    rsl = rsr.tile([1, 2, N], F32, name="rsl", tag="rs")
                if h == H - 1:
                    # ct 0..4 normalizations ahead of the last head: they
                    # fill the PE stall while head 10's qk tile evacuates
                    # (their reciprocals resolved during head 9)
                    for ct in range(CT - 1):
                        emit_norm(ct, psQV, "qv")
                emit_head(h, rsl)
                if h % 2 == 0 and h + 2 < H:
                    emit_qk_tile(h // 2 + 1)
                if h % 2 == 1:
                    emit_pair_recip(h // 2, rsl)

        # ---- phase F+G: last reciprocal chunk, normalize, projection -
        nc.sync.dma_start(out=rs16, in_=rs12[32:34, :])
        nc.vector.reciprocal(out=rc16, in_=rs16)
        nc.gpsimd.dma_start(out=rc12b[32:34, :], in_=rc16)
        with (
            tc.tile_pool(name="psR", bufs=1, space="PSUM") as psR,
            tc.tile_pool(name="psO", bufs=3, space="PSUM") as psO,
        ):
            for nt in range(NT):
                pso = psO.tile([128, C], F32, name="pso", tag="pso")
                for ct in range(CT - 1):
                    lhsT = attnT[:, ct, nt * 128:(nt + 1) * 128]
                    nc.tensor.matmul(pso[:, 0:512], lhsT, pwT[:, ct, 0:512],
                                     start=(ct == 0), stop=False)
                    nc.tensor.matmul(pso[:, 512:768], lhsT, pwT[:, ct, 512:768],
                                     start=(ct == 0), stop=False)
                if nt == 0:
                    # the last head pair's normalization, emitted behind
                    # nt=0's first matmuls so its chain resolves off the
                    # critical path
                    emit_norm(CT - 1, psR, "psr")
                for ct in [CT - 1]:
                    lhsT = attnT[:, ct, nt * 128:(nt + 1) * 128]
                    nc.tensor.matmul(pso[:, 0:512], lhsT, pwT[:, ct, 0:512],
                                     start=False, stop=False)
                    nc.tensor.matmul(pso[:, 512:768], lhsT, pwT[:, ct, 512:768],
                                     start=False, stop=False)
                nc.tensor.matmul(pso[:, 0:512], ones, pb[:, 0:512],
                                 start=False, stop=True)
                nc.tensor.matmul(pso[:, 512:768], ones, pb[:, 512:768],
                                 start=False, stop=True)
                o_sb = work.tile([128, C], F32, name="o_sb", tag="o_sb")
                if nt % 2 == 0:
                    nc.scalar.copy(out=o_sb, in_=pso)
                else:
                    nc.vector.tensor_copy(out=o_sb, in_=pso)
                nc.sync.dma_start(out=out_d.ap()[nt * 128:(nt + 1) * 128, :],
                                  in_=o_sb)


def _host_prep_shared(qkv_w, qkv_b, proj_w, proj_b):
    wT = np.ascontiguousarray(qkv_w.T).astype(NPBF16)          # [C, 3C]
    pwT = np.ascontiguousarray(proj_w.T).astype(NPBF16)        # [C, C]
    qb_col = np.ascontiguousarray(
        qkv_b[:2 * C].reshape(2 * C // 128, 128).T).astype(np.float32)
    vb_row = qkv_b[2 * C:].reshape(1, C).astype(NPBF16)
    pb_row = proj_b.reshape(1, C).astype(NPBF16)
    ones_row = np.ones((1, 128), NPBF16)
    ind = np.zeros((34, C), np.float32)
    for h in range(H):
        r = h if h < 10 else 32 + (h - 10)
        ind[r, h * D:(h + 1) * D] = 1.0
    ind = ind.astype(NPBF16)
    return wT, pwT, qb_col, vb_row, pb_row, ones_row, ind


def kernel(x, mask, qkv_w, qkv_b, proj_w, proj_b, _trace=False):
    if "nc" not in _CACHE:
        _CACHE["nc"] = _build_bass()
    nc = _CACHE["nc"]

    wT, pwT, qb_col, vb_row, pb_row, ones_row, ind = _host_prep_shared(
        np.asarray(qkv_w), np.asarray(qkv_b), np.asarray(proj_w),
        np.asarray(proj_b))
    x = np.asarray(x)
    mask = np.asarray(mask)

    in_maps = []
    for i in range(B):
        in_maps.append({
            "xT": np.ascontiguousarray(x[i].T).astype(NPBF16),
            "maskT": np.ascontiguousarray(mask[i].T).astype(NPBF16),
            "qkv_wT": wT,
            "proj_wT": pwT,
            "qb_col": qb_col,
            "vb_row": vb_row,
            "pb_row": pb_row,
            "ones_row": ones_row,
            "ind": ind,
        })
    res = run_bass_kernel_spmd(nc, in_maps, core_ids=list(range(B)),
                               trace=_trace)
    out = np.stack([res.results[i]["out"] for i in range(B)], axis=0)
    if _trace:
        _CACHE["last_results"] = res
    return out


# revision 53
# speedup vs baseline: 1.0052x; 1.0052x over previous
"""Masked multi-head attention (B=8, N=1024, C=768, H=12) on 8 trn2 NeuronCores.

Sharding: pure data-parallel over batch - core i computes batch element i
end-to-end (qkv linear, masked softmax attention, output projection).
No collectives.

Device-side layout is fully "transposed attention":
  qkT   [2C, N]  (q/k heads as [D, N] blocks, produced directly by matmul)
  S.T   [m, n] per head (keys on partitions)  ->  softmax over partition dim
        handled with: no max-subtraction (scores are O(1)), row-sums via a
        ones-column appended to V in the P@V matmul.
  EV.T  [D+1, N] per head accumulated in PSUM; row D holds the softmax denom.
Host pre-transposes/casts x, mask, qkv_w, proj_w (layout prep for the chosen
sharding); all matmuls run in bf16 with fp32 PSUM accumulation.
"""

import numpy as np
import ml_dtypes

import concourse.bass as bass
import concourse.mybir as mybir
import concourse.tile as tile
from concourse import bacc
from concourse.bass_utils import run_bass_kernel_spmd

B, N, C, H = 8, 1024, 768, 12
D = C // H  # 64
SCALE = 0.125
NT = N // 128  # 8 n-tiles
CT = C // 128  # 6 c-tiles
BF16 = mybir.dt.bfloat16
F32 = mybir.dt.float32
NPBF16 = ml_dtypes.bfloat16

_CACHE: dict = {}


def _build_bass():
    nc = bacc.Bacc(None, target_bir_lowering=False, debug=False)

    xT_d = nc.dram_tensor("xT", [C, N], BF16, kind="ExternalInput")
    maskT_d = nc.dram_tensor("maskT", [N, N], BF16, kind="ExternalInput")
    wT_d = nc.dram_tensor("qkv_wT", [C, 3 * C], BF16, kind="ExternalInput")
    pwT_d = nc.dram_tensor("proj_wT", [C, C], BF16, kind="ExternalInput")
    qb_d = nc.dram_tensor("qb_col", [128, 2 * C // 128], F32, kind="ExternalInput")
    vb_d = nc.dram_tensor("vb_row", [1, C], BF16, kind="ExternalInput")
    pb_d = nc.dram_tensor("pb_row", [1, C], BF16, kind="ExternalInput")
    ones_d = nc.dram_tensor("ones_row", [1, 128], BF16, kind="ExternalInput")
    ind_d = nc.dram_tensor("ind", [34, C], BF16, kind="ExternalInput")
    out_d = nc.dram_tensor("out", [N, C], F32, kind="ExternalOutput")

    with tile.TileContext(nc) as tc:
        _emit(nc, tc, xT_d, maskT_d, wT_d, pwT_d, qb_d, vb_d, pb_d,
              ones_d, ind_d, out_d)
    nc.compile()
    return nc


def _emit(nc, tc, xT_d, maskT_d, wT_d, pwT_d, qb_d, vb_d, pb_d,
          ones_d, ind_d, out_d):
    Exp = mybir.ActivationFunctionType.Exp
    Ident = mybir.ActivationFunctionType.Identity

    with (
        tc.tile_pool(name="consts", bufs=1) as consts,
        tc.tile_pool(name="work", bufs=6) as work,
    ):
        # ---- persistent SBUF residents -------------------------------
        xT = consts.tile([128, CT, N], BF16, name="xT_sb")
        wT = consts.tile([128, CT, 3 * C], BF16, name="wT_sb")
        pwT = consts.tile([128, CT, C], BF16, name="pwT_sb")
        maskT = consts.tile([128, NT, N], BF16, name="maskT_sb")
        qb = consts.tile([128, 2 * C // 128], F32, name="qb_sb")
        vb = consts.tile([1, C], BF16, name="vb_sb")
        pb = consts.tile([1, C], BF16, name="pb_sb")
        ones = consts.tile([1, 128], BF16, name="ones_sb")
        qkT = consts.tile([128, 2 * C // 128, N], BF16, name="qkT_sb")
        v_ext = consts.tile([128, NT, H * (D + 1)], BF16, name="vext_sb")
        attnT = consts.tile([128, CT, N], BF16, name="attnT_sb")
        ind = consts.tile([34, C], BF16, name="ind_sb")
        rs12 = consts.tile([34, N], F32, name="rs12_sb")
        rs80 = consts.tile([80, 128], F32, name="rs80_sb")
        rc80 = consts.tile([80, 128], F32, name="rc80_sb")
        rs40 = consts.tile([40, 128], F32, name="rs40_sb")
        rc40 = consts.tile([40, 128], F32, name="rc40_sb")
        rc12b = consts.tile([34, N], BF16, name="rc12_sb")

        # split loads, issued in consumption order so compute starts early
        xT_r = xT_d.ap().rearrange("(t p) n -> p t n", p=128)
        wT_r = wT_d.ap().rearrange("(t p) n -> p t n", p=128)
        maskT_r = maskT_d.ap().rearrange("(t p) n -> p t n", p=128)
        pwT_r = pwT_d.ap().rearrange("(t p) n -> p t n", p=128)
        # tiny const loads on the ACT HWDGE ring, big loads on the SP ring
        nc.scalar.dma_start(out=qb, in_=qb_d.ap())
        nc.scalar.dma_start(out=vb, in_=vb_d.ap())
        nc.scalar.dma_start(out=ones, in_=ones_d.ap())
        nc.scalar.dma_start(out=pb, in_=pb_d.ap())
        nc.scalar.dma_start(out=ind, in_=ind_d.ap())
        for ct in range(CT):
            nc.sync.dma_start(out=wT[:, ct, :], in_=wT_r[:, ct, :])
            nc.sync.dma_start(out=xT[:, ct, :], in_=xT_r[:, ct, :])
        for j in range(NT):
            nc.sync.dma_start(out=maskT[:, j, :], in_=maskT_r[:, j, :])
        for ct in range(CT):
            nc.sync.dma_start(out=pwT[:, ct, :], in_=pwT_r[:, ct, :])

        nc.vector.memset(rc12b, 0.0)

        # ones columns of v_ext (col D of each head block)
        v_ext_h = v_ext.rearrange("p t (h e) -> p t h e", e=D + 1)
        nc.vector.memset(v_ext_h[:, :, :, D:D + 1], 1.0)

        with (
            tc.tile_pool(name="rsr", bufs=2) as rsr,
            tc.tile_pool(name="psQV", bufs=1, space="PSUM") as psQV,
            tc.tile_pool(name="psST", bufs=2, space="PSUM") as psST,
            tc.tile_pool(name="psEV", bufs=1, space="PSUM") as psEV,
        ):
            def emit_qk_tile(i, pool=None, tag="qv"):
                # qkT rows i*128..: q rows for i<6, k rows for i>=6
                psQ = (pool or psQV).tile([128, N], F32, name="psQ", tag=tag)
                for ct in range(CT):
                    lhsT = wT[:, ct, i * 128:(i + 1) * 128]
                    for half in range(2):
                        nc.tensor.matmul(
                            psQ[:, half * 512:(half + 1) * 512],
                            lhsT,
                            xT[:, ct, half * 512:(half + 1) * 512],
                            start=(ct == 0),
                            stop=(ct == CT - 1),
                        )
                if i < CT:
                    nc.scalar.activation(
                        out=qkT[:, i, :], in_=psQ[:, 0:N], func=Ident,
                        bias=qb[:, i:i + 1], scale=1.0,
                    )
                else:
                    nc.vector.tensor_scalar_add(
                        out=qkT[:, i, :], in0=psQ[:, 0:N],
                        scalar1=qb[:, i:i + 1],
                    )

            def emit_v_tile(nt):
                psV = psQV.tile([128, N], F32, name="psV", tag="qv")
                for ct in range(CT):
                    lhsT = xT[:, ct, nt * 128:(nt + 1) * 128]
                    nc.tensor.matmul(
                        psV[:, 0:512], lhsT, wT[:, ct, 2 * C:2 * C + 512],
                        start=(ct == 0), stop=False,
                    )
                    nc.tensor.matmul(
                        psV[:, 512:768], lhsT, wT[:, ct, 2 * C + 512:3 * C],
                        start=(ct == 0), stop=False,
                    )
                nc.tensor.matmul(psV[:, 0:512], ones, vb[:, 0:512],
                                 start=False, stop=True)
                nc.tensor.matmul(psV[:, 512:768], ones, vb[:, 512:768],
                                 start=False, stop=True)
                nc.vector.tensor_copy(
                    out=v_ext_h[:, nt, :, 0:D],
                    in_=psV[:, 0:C].rearrange("p (h d) -> p h d", d=D),
                )

            def emit_head(h, rsl):
                po = (h % 2) * 64  # partition offset of this head's d-rows
                qt = h // 2        # q tile index; k tile = 6 + qt
                psE = psEV.tile([D + 1, N], F32, name="psE", tag="psE")
                for j in range(NT):
                    if h == 0:
                        emit_v_tile(j)
                    if h % 2 == 1 and h + 2 < H and j == NT - 3:
                        # next pair's k tile, emitted a few j-steps early so
                        # its evacuation clears the DVE queue before the
                        # next head's first S matmul needs it
                        emit_qk_tile(CT + h // 2 + 1)
                    psS = psST.tile([128, N], F32, name="psS", tag="psS")
                    kT_ap = qkT[po:po + D, CT + qt, j * 128:(j + 1) * 128]
                    for half in range(2):
                        nc.tensor.matmul(
                            psS[:, half * 512:(half + 1) * 512],
                            kT_ap,
                            qkT[po:po + D, qt, half * 512:(half + 1) * 512],
                            start=True, stop=True,
                        )
                    e_sb = work.tile([128, N], BF16, name="e_sb", tag="e_sb")
                    nc.scalar.activation(out=e_sb, in_=psS, func=Exp,
                                         scale=SCALE)
                    em_sb = work.tile([128, N], BF16, name="em_sb",
                                      tag="em_sb")
                    nc.vector.tensor_mul(out=em_sb, in0=e_sb,
                                         in1=maskT[:, j, :])
                    v_ap = v_ext[:, j, h * (D + 1):(h + 1) * (D + 1)]
                    for half in range(2):
                        nc.tensor.matmul(
                            psE[:, half * 512:(half + 1) * 512],
                            v_ap,
                            em_sb[:, half * 512:(half + 1) * 512],
                            start=(j == 0), stop=(j == NT - 1),
                        )
                # evacuate: EV rows -> attnT (bf16), denom row -> rs (fp32).
                # The last pair's evacuations go to the (by then idle)
                # scalar engine so the EV pool releases promptly for proj.
                if h >= H - 2:
                    nc.scalar.copy(out=attnT[po:po + D, qt, :],
                                   in_=psE[0:D, :])
                    nc.scalar.copy(out=rsl[:, h % 2, :], in_=psE[D:D + 1, :])
                else:
                    nc.vector.tensor_copy(out=attnT[po:po + D, qt, :],
                                          in_=psE[0:D, :])
                    nc.vector.tensor_copy(out=rsl[:, h % 2, :],
                                          in_=psE[D:D + 1, :])

            def emit_norm(ct, pool, tag):
                # broadcast the pair's reciprocal rows onto 2x64 partitions
                # via an indicator matmul, then scale attnT in place
                psr = pool.tile([128, N], F32, name="psr", tag=tag)
                for half in range(2):
                    nc.tensor.matmul(
                        psr[:, half * 512:(half + 1) * 512],
                        ind[:, ct * 128:(ct + 1) * 128],
                        rc12b[:, half * 512:(half + 1) * 512],
                        start=True, stop=True,
                    )
                nc.vector.tensor_mul(out=attnT[:, ct, :], in0=attnT[:, ct, :],
                                     in1=psr)

            def emit_pair_recip(hp, rsl):
                # spread the pair's two denominator rows (partition 0) onto
                # partitions 2hp,2hp+1 of rs12 (pair 5 goes to partitions
                # 32,33 so its reciprocal is a legal standalone ACT op)
                ro = 2 * hp if hp < 5 else 32
                nc.sync.dma_start(out=rs12[ro:ro + 2, :], in_=rsl)
                if hp == 4:
                    # heads 0..9 are all in rs12: reciprocal on DVE at 8x
                    # lane efficiency by spreading [10, N] onto [80, 128]
                    # (SBUF->SBUF partition-spread DMA; same flat order).
                    # No ACT involvement -> no activation-table switches.
                    nc.sync.dma_start(out=rs80, in_=rs12[0:10, :])
                    nc.vector.reciprocal(out=rc80, in_=rs80)
                    nc.gpsimd.dma_start(out=rc12b[0:10, :], in_=rc80)

            # pair 0's qk tiles first, then heads with the next pair's qk
            # tiles emitted between the two heads of each pair so the
            # scheduler can fill attention's ACT-gated PE gaps with them
            # pair-0 qk tiles borrow the two (still idle) ST slots so they
            # pipeline with the arriving weight/x loads
            emit_qk_tile(0, pool=psST, tag="psS")
            emit_qk_tile(CT, pool=psST, tag="psS")
            for h in range(H):
                if h % 2 == 0:
                    rsl = rsr.tile([1, 2, N], F32, name="rsl", tag="rs")
                if h == H - 1:
                    # ct 0..4 normalizations ahead of the last head: they
                    # fill the PE stall while head 10's qk tile evacuates
                    # (their reciprocals resolved during head 9)
                    for ct in range(CT - 1):
                        emit_norm(ct, psQV, "qv")
                emit_head(h, rsl)
                if h % 2 == 0 and h + 2 < H:
                    emit_qk_tile(h // 2 + 1)
                if h == H - 2:
                    nc.sync.dma_start(out=rs40[0:8, :], in_=rsl[:, 0, :])
                    nc.vector.reciprocal(out=rc40[0:8, :], in_=rs40[0:8, :])
                    nc.gpsimd.dma_start(out=rc12b[32:33, :], in_=rc40[0:8, :])
                if h % 2 == 1:
                    emit_pair_recip(h // 2, rsl)

        # ---- phase F+G: last reciprocal chunk, normalize, projection -
        nc.sync.dma_start(out=rs40[32:40, :], in_=rs12[33:34, :])
        nc.vector.reciprocal(out=rc40[32:40, :], in_=rs40[32:40, :])
        nc.gpsimd.dma_start(out=rc12b[33:34, :], in_=rc40[32:40, :])
        with (
            tc.tile_pool(name="psR", bufs=1, space="PSUM") as psR,
            tc.tile_pool(name="psO", bufs=3, space="PSUM") as psO,
        ):
            for nt in range(NT):
                pso = psO.tile([128, C], F32, name="pso", tag="pso")
                for ct in range(CT - 1):
                    lhsT = attnT[:, ct, nt * 128:(nt + 1) * 128]
                    nc.tensor.matmul(pso[:, 0:512], lhsT, pwT[:, ct, 0:512],
                                     start=(ct == 0), stop=False)
                    nc.tensor.matmul(pso[:, 512:768], lhsT, pwT[:, ct, 512:768],
                                     start=(ct == 0), stop=False)
                if nt == 0:
                    # the last head pair's normalization, emitted behind
                    # nt=0's first matmuls so its chain resolves off the
                    # critical path
                    emit_norm(CT - 1, psR, "psr")
                for ct in [CT - 1]:
                    lhsT = attnT[:, ct, nt * 128:(nt + 1) * 128]
                    nc.tensor.matmul(pso[:, 0:512], lhsT, pwT[:, ct, 0:512],
                                     start=False, stop=False)
                    nc.tensor.matmul(pso[:, 512:768], lhsT, pwT[:, ct, 512:768],
                                     start=False, stop=False)
                nc.tensor.matmul(pso[:, 0:512], ones, pb[:, 0:512],
                                 start=False, stop=True)
                nc.tensor.matmul(pso[:, 512:768], ones, pb[:, 512:768],
                                 start=False, stop=True)
                o_sb = work.tile([128, C], F32, name="o_sb", tag="o_sb")
                if nt % 2 == 0:
                    nc.scalar.copy(out=o_sb, in_=pso)
                else:
                    nc.vector.tensor_copy(out=o_sb, in_=pso)
                nc.sync.dma_start(out=out_d.ap()[nt * 128:(nt + 1) * 128, :],
                                  in_=o_sb)


def _host_prep_shared(qkv_w, qkv_b, proj_w, proj_b):
    wT = np.ascontiguousarray(qkv_w.T).astype(NPBF16)          # [C, 3C]
    pwT = np.ascontiguousarray(proj_w.T).astype(NPBF16)        # [C, C]
    qb_col = np.ascontiguousarray(
        qkv_b[:2 * C].reshape(2 * C // 128, 128).T).astype(np.float32)
    vb_row = qkv_b[2 * C:].reshape(1, C).astype(NPBF16)
    pb_row = proj_b.reshape(1, C).astype(NPBF16)
    ones_row = np.ones((1, 128), NPBF16)
    ind = np.zeros((34, C), np.float32)
    for h in range(H):
        r = h if h < 10 else 32 + (h - 10)
        ind[r, h * D:(h + 1) * D] = 1.0
    ind = ind.astype(NPBF16)
    return wT, pwT, qb_col, vb_row, pb_row, ones_row, ind


def kernel(x, mask, qkv_w, qkv_b, proj_w, proj_b, _trace=False):
    if "nc" not in _CACHE:
        _CACHE["nc"] = _build_bass()
    nc = _CACHE["nc"]

    wT, pwT, qb_col, vb_row, pb_row, ones_row, ind = _host_prep_shared(
        np.asarray(qkv_w), np.asarray(qkv_b), np.asarray(proj_w),
        np.asarray(proj_b))
    x = np.asarray(x)
    mask = np.asarray(mask)

    in_maps = []
    for i in range(B):
        in_maps.append({
            "xT": np.ascontiguousarray(x[i].T).astype(NPBF16),
            "maskT": np.ascontiguousarray(mask[i].T).astype(NPBF16),
            "qkv_wT": wT,
            "proj_wT": pwT,
            "qb_col": qb_col,
            "vb_row": vb_row,
            "pb_row": pb_row,
            "ones_row": ones_row,
            "ind": ind,
        })
    res = run_bass_kernel_spmd(nc, in_maps, core_ids=list(range(B)),
                               trace=_trace)
    out = np.stack([res.results[i]["out"] for i in range(B)], axis=0)
    if _trace:
        _CACHE["last_results"] = res
    return out
